# revision 2
# baseline (speedup 1.0000x reference)
"""MetricLoss kernel for 8 Trainium2 NeuronCores (Bass/Tile).

Problem: x [B=1024, M=32, F=256] f32; per-part pairwise squared distances
d[i,j,m] = ||x[i,m]-x[j,m]||^2; groups of K=4 consecutive rows;
  loss_homo  = 2/(B(K-1))   * sum_{same group, i<j, m} d
  loss_heter = 2/(B(B-K))   * sum_{group_i<group_j, m} relu(1-d)
Returns np.float32 [2] = (loss_homo, loss_heter).

Strategy (one identical NEFF on 8 cores, per-core DATA differs):
- Host computes sq_i = ||x[i,m]||^2 and builds augmented fp16 operands so a
  3-matmul PSUM accumulation produces d' = d - 2*SQ_SHIFT directly:
    lhsT = [-2*x ; 1 ; sq_i-S],  rhs = [x ; sq_j-S ; 1]   (K = 256+2)
- Symmetry halving via cyclic panels: core c owns row-slab c (128 rows) and
  processes column slabs c..c+4 (mod 8). Distance-1..3 block sums count
  double (they also stand for their mirrored distance-5..7 blocks),
  distance-4 counts once, diagonal-slab blocks are mask-corrected on-core.
- ACT does relu(1-d) free-dim accumulation on panels 1-4; DVE handles the
  diagonal panel with masks:  relu(1-d) = -min(d'+BIG, 0)  (exact).
- Per-core outputs are [128,128] f32 partial sums; host reduces in float64.
"""

import numpy as np

B = 1024
M = 32
F = 256
KG = 4  # group size
NSLAB = 8
SLAB = 128
NPANEL = 5  # own slab + next 4 (cyclic)
NA = 512  # panels 0-3 -> PSUM tile A
NB = 128  # panel 4    -> PSUM tile B
SQ_SHIFT = 256.0
RELU_BIAS = 1.0 - 2.0 * SQ_SHIFT  # relu(1-d) = relu(-d' + RELU_BIAS)
MBLK = 8  # m-values per rhs DMA block
NBLK = M // MBLK

_CACHE = {}


def _build_nc():
    from concourse import bacc
    import concourse.mybir as mybir
    import concourse.tile as tile

    nc = bacc.Bacc("TRN2", target_bir_lowering=False, debug=False, num_devices=8)
    f16, f32 = mybir.dt.float16, mybir.dt.float32
    Relu = mybir.ActivationFunctionType.Relu
    mult, add, amin = (
        mybir.AluOpType.mult,
        mybir.AluOpType.add,
        mybir.AluOpType.min,
    )

    rhs0_d = nc.dram_tensor("rhs0", [SLAB, M, NPANEL * SLAB], f16, kind="ExternalInput")
    rhs1_d = nc.dram_tensor("rhs1", [SLAB, M, NPANEL * SLAB], f16, kind="ExternalInput")
    rhsa_d = nc.dram_tensor("rhsa", [2, M, NPANEL * SLAB], f16, kind="ExternalInput")
    lhs0_d = nc.dram_tensor("lhs0", [SLAB, M, SLAB], f16, kind="ExternalInput")
    lhs1_d = nc.dram_tensor("lhs1", [SLAB, M, SLAB], f16, kind="ExternalInput")
    lhsa_d = nc.dram_tensor("lhsa", [2, M, SLAB], f16, kind="ExternalInput")
    mcross_d = nc.dram_tensor("mcross", [SLAB, SLAB], f32, kind="ExternalInput")
    msg_d = nc.dram_tensor("msg", [SLAB, SLAB], f32, kind="ExternalInput")
    bias_d = nc.dram_tensor("bias", [SLAB, 1], f32, kind="ExternalInput")
    out_d = nc.dram_tensor("out", [SLAB, 4 * M], f32, kind="ExternalOutput")

    with tile.TileContext(nc) as tc:
        with (
            tc.tile_pool(name="res", bufs=1) as res,
            tc.tile_pool(name="scr", bufs=3) as scr,
            tc.tile_pool(name="psa", bufs=3, space="PSUM") as psa,
            tc.tile_pool(name="psb", bufs=3, space="PSUM") as psb,
        ):
            # Small resident inputs first (needed by m=0).
            lhs0_t = res.tile([SLAB, M, SLAB], f16)
            lhs1_t = res.tile([SLAB, M, SLAB], f16)
            lhsa_t = res.tile([2, M, SLAB], f16)
            rhsa_t = res.tile([2, M, NPANEL * SLAB], f16)
            mcross_t = res.tile([SLAB, SLAB], f32)
            msg_t = res.tile([SLAB, SLAB], f32)
            bias_t = res.tile([SLAB, 1], f32)
            nc.sync.dma_start(out=lhs0_t, in_=lhs0_d[:, :, :])
            nc.sync.dma_start(out=lhs1_t, in_=lhs1_d[:, :, :])
            nc.sync.dma_start(out=lhsa_t, in_=lhsa_d[:, :, :])
            nc.sync.dma_start(out=rhsa_t, in_=rhsa_d[:, :, :])
            nc.sync.dma_start(out=mcross_t, in_=mcross_d[:, :])
            nc.sync.dma_start(out=msg_t, in_=msg_d[:, :])
            nc.sync.dma_start(out=bias_t, in_=bias_d[:, :])

            # Big rhs panels, blocked by m for DMA/compute overlap.
            rhs0_bt = []
            rhs1_bt = []
            for b in range(NBLK):
                t0 = res.tile(
                    [SLAB, MBLK, NPANEL * SLAB], f16, name=f"rhs0b{b}", tag=f"rhs0b{b}"
                )
                t1 = res.tile(
                    [SLAB, MBLK, NPANEL * SLAB], f16, name=f"rhs1b{b}", tag=f"rhs1b{b}"
                )
                nc.sync.dma_start(
                    out=t0, in_=rhs0_d[:, b * MBLK : (b + 1) * MBLK, :]
                )
                nc.sync.dma_start(
                    out=t1, in_=rhs1_d[:, b * MBLK : (b + 1) * MBLK, :]
                )
                rhs0_bt.append(t0)
                rhs1_bt.append(t1)

            accU = res.tile([SLAB, M], f32)
            accV = res.tile([SLAB, M], f32)
            accH = res.tile([SLAB, M], f32)
            accS = res.tile([SLAB, M], f32)
            zero_t = res.tile([SLAB, NB], f32)
            nc.vector.memset(zero_t, 0.0)

            # ACT warm-up: absorb the bias DMA wait + table load early.
            act_warm = res.tile([SLAB, 1], f32)
            nc.scalar.activation(
                out=act_warm, in_=bias_t, func=Relu, bias=bias_t[:, 0:1], scale=0.0
            )

            for m in range(M):
                b, mm = divmod(m, MBLK)
                r0_m = rhs0_bt[b][:, mm, :]
                r1_m = rhs1_bt[b][:, mm, :]
                ra_m = rhsa_t[:, m, :]
                l0_m = lhs0_t[:, m, :]
                l1_m = lhs1_t[:, m, :]
                la_m = lhsa_t[:, m, :]

                psA = psa.tile([SLAB, NA], f32)
                psB = psb.tile([SLAB, NB], f32)
                nc.tensor.matmul(psA, l0_m, r0_m[:, 0:NA], start=True, stop=False)
                nc.tensor.matmul(psA, l1_m, r1_m[:, 0:NA], start=False, stop=False)
                nc.tensor.matmul(psA, la_m, ra_m[:, 0:NA], start=False, stop=True)
                nc.tensor.matmul(
                    psB, l0_m, r0_m[:, NA : NA + NB], start=True, stop=False
                )
                nc.tensor.matmul(
                    psB, l1_m, r1_m[:, NA : NA + NB], start=False, stop=False
                )
                nc.tensor.matmul(
                    psB, la_m, ra_m[:, NA : NA + NB], start=False, stop=True
                )

                # ACT: unmasked relu(1-d) row-sums for panels 1-3 and panel 4.
                junkA = scr.tile([SLAB, NA - NB], f16)
                nc.scalar.activation(
                    out=junkA,
                    in_=psA[:, NB:NA],
                    func=Relu,
                    bias=bias_t[:, 0:1],
                    scale=-1.0,
                    accum_out=accU[:, m : m + 1],
                )
                junkB = scr.tile([SLAB, NB], f16)
                nc.scalar.activation(
                    out=junkB,
                    in_=psB,
                    func=Relu,
                    bias=bias_t[:, 0:1],
                    scale=-1.0,
                    accum_out=accV[:, m : m + 1],
                )

                # DVE: diagonal panel. r0 = min(d'+BIG, 0) = -relu(1-d).
                r0 = scr.tile([SLAB, NB], f32)
                nc.vector.scalar_tensor_tensor(
                    out=r0,
                    in0=psA[:, 0:NB],
                    scalar=-RELU_BIAS,
                    in1=zero_t,
                    op0=add,
                    op1=amin,
                )
                junkH = scr.tile([SLAB, NB], f32)
                dedH = scr.tile([SLAB, 1], f32)
                nc.vector.scalar_tensor_tensor(
                    out=junkH,
                    in0=r0,
                    scalar=1.0,
                    in1=mcross_t,
                    op0=mult,
                    op1=mult,
                    accum_out=dedH[:, 0:1],
                )
                nc.vector.tensor_copy(accH[:, m : m + 1], dedH)
                junkS = scr.tile([SLAB, NB], f32)
                dedS = scr.tile([SLAB, 1], f32)
                nc.vector.scalar_tensor_tensor(
                    out=junkS,
                    in0=psA[:, 0:NB],
                    scalar=1.0,
                    in1=msg_t,
                    op0=mult,
                    op1=mult,
                    accum_out=dedS[:, 0:1],
                )
                nc.vector.tensor_copy(accS[:, m : m + 1], dedS)

            nc.sync.dma_start(out=out_d[:, 0 * M : 1 * M], in_=accU)
            nc.sync.dma_start(out=out_d[:, 1 * M : 2 * M], in_=accV)
            nc.sync.dma_start(out=out_d[:, 2 * M : 3 * M], in_=accH)
            nc.sync.dma_start(out=out_d[:, 3 * M : 4 * M], in_=accS)
    nc.compile()
    return nc


def _prep_inputs(x):
    """Build the 8 per-core input dicts from full x [B, M, F] f32."""
    x = np.asarray(x, dtype=np.float32)
    assert x.shape == (B, M, F), x.shape
    sq = (x.astype(np.float64) ** 2).sum(-1)  # [B, M]
    sqs16 = (sq - SQ_SHIFT).astype(np.float16)  # [B, M] shifted, fp16
    xt = np.ascontiguousarray(x.transpose(2, 1, 0))  # [F, M, B]
    xt16 = xt.astype(np.float16)
    ones_m = np.ones((M,), np.float16)

    # Masks: within the 128-row diagonal block, group structure is
    # position-invariant across cores (groups of 4 consecutive rows).
    p = np.arange(SLAB)
    same = (p[:, None] // KG) == (p[None, :] // KG)
    mcross = (~same).astype(np.float32)
    msg = (same & (p[:, None] != p[None, :])).astype(np.float32)
    bias = np.full((SLAB, 1), RELU_BIAS, np.float32)

    in_maps = []
    for c in range(NSLAB):
        cols = np.concatenate(
            [np.arange(SLAB) + SLAB * ((c + t) % NSLAB) for t in range(NPANEL)]
        )
        own = cols[0:SLAB]
        rhs0 = np.ascontiguousarray(xt16[0:SLAB][:, :, cols])
        rhs1 = np.ascontiguousarray(xt16[SLAB:F][:, :, cols])
        rhsa = np.ascontiguousarray(
            np.stack(
                [
                    sqs16[cols, :].T,  # [M, 640]: sq_j - S
                    np.broadcast_to(ones_m[:, None], (M, NPANEL * SLAB)),
                ]
            )
        )
        lhs0 = np.ascontiguousarray(-2.0 * xt16[0:SLAB][:, :, own].astype(np.float32)).astype(np.float16)
        lhs1 = np.ascontiguousarray(-2.0 * xt16[SLAB:F][:, :, own].astype(np.float32)).astype(np.float16)
        lhsa = np.ascontiguousarray(
            np.stack(
                [
                    np.broadcast_to(ones_m[:, None], (M, SLAB)),
                    sqs16[own, :].T,  # [M, 128]: sq_i - S
                ]
            )
        )
        in_maps.append(
            {
                "rhs0": rhs0,
                "rhs1": rhs1,
                "rhsa": rhsa,
                "lhs0": lhs0,
                "lhs1": lhs1,
                "lhsa": lhsa,
                "mcross": mcross,
                "msg": msg,
                "bias": bias,
            }
        )
    return in_maps


def _combine(results):
    """float64 reduction of per-core [128, 4*M] partials -> [2] f32."""
    U = V = Hraw = Sraw = 0.0
    for c in range(NSLAB):
        o = results[c]["out"].astype(np.float64)
        U += o[:, 0 * M : 1 * M].sum()
        V += o[:, 1 * M : 2 * M].sum()
        Hraw += o[:, 2 * M : 3 * M].sum()  # = -sum relu on diag panels
        Sraw += o[:, 3 * M : 4 * M].sum()  # = sum msg * d'
    hd0 = -Hraw
    heter_ordered = 2.0 * U + V + hd0
    n_sg_ordered = B * (KG - 1) * M  # same-group ordered pairs (i != j), all m
    sg_d = Sraw + 2.0 * SQ_SHIFT * n_sg_ordered
    loss_homo = sg_d / (B * (KG - 1))
    loss_heter = heter_ordered / (B * (B - KG))
    return np.array([loss_homo, loss_heter], dtype=np.float32)


def _get_runner():
    """Build (once) a cached jitted 8-core executor for the Bass module.

    Mirrors concourse.bass2jax.run_bass_via_pjrt's multi-core path, but keeps
    the jitted callable so repeat invocations skip retracing/recompiling.
    """
    if "runner" in _CACHE:
        return _CACHE["runner"]
    import jax
    import concourse.mybir as mybir
    from concourse import bass2jax
    from jax.experimental.shard_map import shard_map
    from jax.sharding import Mesh, PartitionSpec

    if "nc" not in _CACHE:
        _CACHE["nc"] = _build_nc()
    nc = _CACHE["nc"]
    bass2jax.install_neuronx_cc_hook()

    partition_name = (
        nc.partition_id_tensor.name if nc.partition_id_tensor else None
    )
    in_names, out_names, out_avals, zero_shapes = [], [], [], []
    for alloc in nc.m.functions[0].allocations:
        if not isinstance(alloc, mybir.MemoryLocationSet):
            continue
        name = alloc.memorylocations[0].name
        if alloc.kind == "ExternalInput":
            if name != partition_name:
                in_names.append(name)
        elif alloc.kind == "ExternalOutput":
            shape = tuple(alloc.tensor_shape)
            dtype = mybir.dt.np(alloc.dtype)
            out_names.append(name)
            out_avals.append(jax.core.ShapedArray(shape, dtype))
            zero_shapes.append((shape, dtype))
    n_params = len(in_names)
    all_names = in_names + out_names
    if partition_name is not None:
        all_names = all_names + [partition_name]
    donate = tuple(range(n_params, n_params + len(out_names)))

    def _body(*args):
        operands = list(args)
        if partition_name is not None:
            operands.append(bass2jax.partition_id_tensor())
        outs = bass2jax._bass_exec_p.bind(
            *operands,
            out_avals=tuple(out_avals),
            in_names=tuple(all_names),
            out_names=tuple(out_names),
            lowering_input_output_aliases=(),
            sim_require_finite=True,
            sim_require_nnan=True,
            nc=nc,
        )
        return tuple(outs)

    devices = jax.devices()[:NSLAB]
    mesh = Mesh(np.asarray(devices), ("core",))
    in_specs = (PartitionSpec("core"),) * (n_params + len(out_names))
    out_specs = (PartitionSpec("core"),) * len(out_names)
    sharded = jax.jit(
        shard_map(
            _body, mesh=mesh, in_specs=in_specs, out_specs=out_specs, check_rep=False
        ),
        donate_argnums=donate,
        keep_unused=True,
    )

    def runner(in_maps):
        concat_in = [
            np.concatenate([in_maps[c][name] for c in range(NSLAB)], axis=0)
            for name in in_names
        ]
        zeros = [
            np.zeros((NSLAB * s[0], *s[1:]), dt) for (s, dt) in zero_shapes
        ]
        out_arrs = sharded(*concat_in, *zeros)
        return [
            {
                name: np.asarray(out_arrs[i]).reshape(
                    NSLAB, *out_avals[i].shape
                )[c]
                for i, name in enumerate(out_names)
            }
            for c in range(NSLAB)
        ]

    runner.sharded = sharded
    runner.in_names = in_names
    runner.zero_shapes = zero_shapes
    runner.out_names = out_names
    runner.out_avals = out_avals
    runner.mesh = mesh
    _CACHE["runner"] = runner
    return runner


def kernel(x, _perf_out=None):
    runner = _get_runner()
    in_maps = _prep_inputs(x)
    results = runner(in_maps)
    return _combine(results)


if __name__ == "__main__":
    rng = np.random.default_rng(0)
    x = rng.standard_normal((B, M, F)).astype(np.float32)
    print(kernel(x))
